# revision 24
# baseline (speedup 1.0000x reference)
"""Trainium2 Bass kernel for nn_AttentionStem (5x5 local attention stem, stride 2).

Self-contained: hardcodes shapes B=8, CIN=64, H=W=128, OUT_CH=128, M=2, K=5.
Data-parallel over batch: one batch element per NeuronCore (8 cores).

Math (per batch):
  scores[k,(h,w)] = x_s(2h,2w)^T G x(p'_k),  G = w_q^T w_k   (q/k projections folded)
  attn = softmax_k(scores)
  out[c,(h,w)] = sum_k attn_k sum_m wpos[m,k] v[2c+m, p'_k],  v = w_v x

Layout tricks vs v1:
  - x stored bf16 with even image rows on partitions 0:64 and odd rows on
    64:128 -> V/ST matmuls (K=64) run as two concurrent PE row-tiles.
  - y duplicated on both partition halves via lhsT=[G|G].
  - score slabs trimmed to 896 cols (t=1 row only feeds one pair).
  - elementwise work split between ACT and DVE.
"""

import os
import sys

for _p in ("/opt/pypackages", "/opt/trn_rl_repo"):
    if _p not in sys.path:
        sys.path.insert(0, _p)

from contextlib import ExitStack

import ml_dtypes
import numpy as np

import concourse.bacc as bacc
import concourse.bass as bass
import concourse.mybir as mybir
from concourse.bass_utils import run_bass_kernel_spmd
from concourse.tile import TileContext

F32 = mybir.dt.float32
BF16 = mybir.dt.bfloat16

NCORES = 8
CIN = 64
IMG = 128          # input H = W
OC = 128           # out channels
VCH = 258          # V row pitch: 256 v-channels + 2 ones columns
VROWS = 132        # 2 pad + 128 + 2 pad rows in v_sb
HO = 64            # output H = W
NPAIR = 32         # output row pairs
SLABW = 896        # trimmed transposed-score slab width

# d (= key row r - 4j for pair j) -> col offset of its 128-col block in a slab.
# Blocks from even key rows (PE row-tile 0) sit in PSUM bank A (cols 0:512),
# odd-row blocks (tile 1) in bank B (cols 512:896) — concurrent row-tiled
# matmuls writing the same PSUM bank hang TRN2.
OFF_OF_D = {4: 0, 0: 128, 2: 256, -2: 384, 1: 512, 3: 640, -1: 768}

APPLY_COLSPLIT = os.environ.get("APPLY_COLSPLIT", "0") == "1"


def make_wpos(row_emb, col_emb, mix_emb):
    a = mix_emb.T.astype(np.float64) @ row_emb.astype(np.float64)  # [2,5]
    b = mix_emb.T.astype(np.float64) @ col_emb.astype(np.float64)  # [2,5]
    wp = a[:, :, None] + b[:, None, :]                             # [2,5,5]
    wp = wp - wp.max(axis=0, keepdims=True)
    e = np.exp(wp)
    wp = e / e.sum(axis=0, keepdims=True)
    return wp.reshape(2, 25).astype(np.float32)                    # [m, dh*5+dw]


def make_masks(wpos):
    """wpos-weighted band masks in the trimmed ST layout.

    Returns [128 (kcol), 2 (m), 896] f32; block at OFF_OF_D[d] holds the
    masks for key row r = 4j + d of pair j, cols rho*64 + w."""
    wm = np.zeros((128, 2, SLABW), np.float32)
    for d, base in OFF_OF_D.items():
        for rho in (0, 1):
            dh = d + 2 - 2 * rho
            if not 0 <= dh < 5:
                continue
            for w in range(64):
                for dw in range(5):
                    kc = 2 * w + dw - 2
                    if 0 <= kc < 128:
                        wm[kc, :, base + rho * 64 + w] = wpos[:, dh * 5 + dw]
    return wm


def make_oob():
    """#window entries with out-of-image column, per position in a pair."""
    oob = np.zeros((128, 1), np.float32)
    for rho in (0, 1):
        for w in range(64):
            cnt = sum(1 for dw in range(5) if not 0 <= 2 * w + dw - 2 < 128)
            oob[rho * 64 + w, 0] = 5.0 * cnt
    return oob


def _ap(t, off, dims, p0=0, pn=None):
    a = t[:]
    np_ = pn if pn is not None else a.ap[0][1]
    return bass.AP(tensor=a.tensor, offset=off + p0 * a.ap[0][0],
                   ap=[[a.ap[0][0], np_]] + [list(d) for d in dims])


def build_nc():
    nc = bacc.Bacc("TRN2", target_bir_lowering=False, debug=False, num_devices=NCORES)

    xe_d = nc.dram_tensor("xe", [CIN, 64, IMG], BF16, kind="ExternalInput")
    xo_d = nc.dram_tensor("xo", [CIN, 64, IMG], BF16, kind="ExternalInput")
    g2_d = nc.dram_tensor("g2", [CIN, 128], BF16, kind="ExternalInput")
    wvt_d = nc.dram_tensor("wvt", [128, 256], BF16, kind="ExternalInput")
    wm_d = nc.dram_tensor("wmask", [128, 2 * SLABW], BF16, kind="ExternalInput")
    oob_d = nc.dram_tensor("oob", [128, 1], F32, kind="ExternalInput")
    out_d = nc.dram_tensor("out", [HO * HO, OC], F32, kind="ExternalOutput")

    EXP = mybir.ActivationFunctionType.Exp
    COPY = mybir.ActivationFunctionType.Copy

    with TileContext(nc) as tc, ExitStack() as ctx:
        sg = ctx.enter_context(tc.tile_pool(name="singles", bufs=1))
        # x: partitions 0:64 even image rows, 64:128 odd rows; 64 rows x 128 cols
        x_sb = sg.tile([128, 64 * IMG], BF16)
        v_sb = sg.tile([128, VROWS * VCH], BF16)     # V + ones cols, padded rows
        y_sb = sg.tile([128, 4096], BF16)            # queries, dup on both halves
        wm_sb = sg.tile([128, 2 * SLABW], BF16)
        oob_sb = sg.tile([128, 1], F32)
        g2_sb = sg.tile([64, 128], BF16)
        wvt_sb = sg.tile([128, 256], BF16)

        # g2 first (gates the y prologue); bulky constants after the x load
        nc.sync.dma_start(out=g2_sb[:], in_=g2_d.ap())

        # x load: 8 chunks of 8 packed rows per half, spread over 3 queues
        qs = [nc.gpsimd, nc.sync, nc.scalar]
        for c8 in range(8):
            dst_e = _ap(x_sb, c8 * 8 * IMG, [[1, 8 * IMG]], 0, 64)
            dst_o = _ap(x_sb, c8 * 8 * IMG, [[1, 8 * IMG]], 64, 64)
            src_e = xe_d.ap()[:, c8 * 8:(c8 + 1) * 8, :]
            src_o = xo_d.ap()[:, c8 * 8:(c8 + 1) * 8, :]
            qs[(2 * c8) % 3].dma_start(out=dst_e, in_=src_e)
            qs[(2 * c8 + 1) % 3].dma_start(out=dst_o, in_=src_o)

        nc.sync.dma_start(out=wvt_sb[:], in_=wvt_d.ap())
        nc.sync.dma_start(out=wm_sb[:], in_=wm_d.ap())
        nc.sync.dma_start(out=oob_sb[:], in_=oob_d.ap())

        # V pad rows (zero) + ones columns
        nc.vector.memset(_ap(v_sb, 0, [[1, 2 * VCH]]), 0.0)
        nc.vector.memset(_ap(v_sb, 130 * VCH, [[1, 2 * VCH]]), 0.0)
        nc.vector.memset(_ap(v_sb, 256, [[VCH, VROWS], [1, 2]]), 1.0)

        def xrow(r):
            # key row r: [64 partitions (channels), 128 cols] on its parity half
            p = (r & 1) * 64
            return x_sb[p:p + 64, (r >> 1) * IMG:(r >> 1) * IMG + IMG]

        # ---- fused pipeline: y prologue, then V(s+3) + ST(s) + apply(s-1)
        # per loop iteration, keeping the PE queue dense (HAM stays warm).
        with tc.tile_pool(name="big", bufs=3, space="PSUM") as big, \
             tc.tile_pool(name="aps", bufs=2, space="PSUM") as aps, \
             tc.tile_pool(name="e2t", bufs=4) as e2t, \
             tc.tile_pool(name="a0p", bufs=5) as a0p, \
             tc.tile_pool(name="a1p", bufs=5) as a1p, \
             tc.tile_pool(name="outsb", bufs=4) as outsb, \
             tc.tile_pool(name="dens", bufs=6) as dens:

            A = {}

            # HAM pre-warm: the input-DMA head leaves the PE idle, so real
            # work would start at the cold 1.2 GHz clock.  Burn the wait on
            # dummy matmuls (g2 arrives first) so the clock-gate opens before
            # the y/V/score pipeline begins.
            warm = big.tile([128, 1024], F32, tag="big")
            for _ in range(36):
                nc.tensor.matmul(warm[:, 0:128], g2_sb[:], g2_sb[0:64, 0:128],
                                 start=True, stop=True)

            def make_y(ch):
                yp = big.tile([128, 1024], F32, tag="big")
                for i in range(2):
                    hs0 = ch * 16 + i * 8
                    rhs = _ap(x_sb, hs0 * IMG, [[IMG, 8], [2, 64]], 0, 64)
                    nc.tensor.matmul(yp[:, i * 512:(i + 1) * 512], g2_sb[:],
                                     rhs, start=True, stop=True)
                if ch % 2 == 0:
                    nc.scalar.copy(y_sb[:, ch * 1024:(ch + 1) * 1024], yp[:])
                else:
                    nc.vector.tensor_copy(y_sb[:, ch * 1024:(ch + 1) * 1024], yp[:])

            VP = {}

            def make_v_mms(vs):
                vp = big.tile([128, 1024], F32, tag="big")
                VP[vs] = vp
                # even rows -> bank A (cols 0:512), odd rows -> bank B:
                # concurrent row-tiles must not write the same PSUM bank.
                # Emit rows in order (alternating tiles) so pairs overlap.
                for r_i, q in ((0, 0), (1, 2), (2, 1), (3, 3)):
                    r = vs * 4 + r_i
                    h = (r_i & 1) * 64
                    nc.tensor.matmul(vp[:, q * 256:(q + 1) * 256], xrow(r),
                                     wvt_sb[h:h + 64, :], start=True, stop=True)
            def make_v_copy(vs):
                vp = VP.pop(vs)
                # dst rows in order (r0, r2, r1, r3) to match quarter layout
                dst = _ap(v_sb, (vs * 4 + 2) * VCH,
                          [[VCH, 2], [2 * VCH, 2], [1, 256]])
                src = vp[:].rearrange("p (r c) -> p r c", c=256)
                if vs % 2 == 0:
                    nc.scalar.copy(dst, src)
                else:
                    nc.vector.tensor_copy(dst, src)

            def make_slab(s):
                stp = big.tile([128, 1024], F32, tag="big")
                n = min(2, NPAIR - s) * 128
                # t=0 (tile0, bank A): pairs s-1, s -> cols 0:256
                if s == 0:
                    nc.tensor.matmul(stp[:, 128:256], xrow(0),
                                     y_sb[0:64, 0:128], start=True, stop=True)
                else:
                    nc.tensor.matmul(stp[:, 0:256], xrow(4 * s),
                                     y_sb[0:64, (s - 1) * 128:(s + 1) * 128],
                                     start=True, stop=True)
                # t=1 (tile1, bank B): pair s -> cols 512:640
                nc.tensor.matmul(stp[:, 512:640], xrow(4 * s + 1),
                                 y_sb[64:128, s * 128:(s + 1) * 128],
                                 start=True, stop=True)
                # t=2 (tile0, bank A): pairs s, s+1 -> cols 256:256+n
                nc.tensor.matmul(stp[:, 256:256 + n], xrow(4 * s + 2),
                                 y_sb[0:64, s * 128:s * 128 + n],
                                 start=True, stop=True)
                # t=3 (tile1, bank B): pairs s, s+1 -> cols 640:640+n
                nc.tensor.matmul(stp[:, 640:640 + n], xrow(4 * s + 3),
                                 y_sb[64:128, s * 128:s * 128 + n],
                                 start=True, stop=True)
                e2 = e2t.tile([128, SLABW], BF16)
                nc.scalar.activation(out=e2[:], in_=stp[:, 0:SLABW], func=EXP)
                a01 = a0p.tile([128, 2 * SLABW], BF16)
                e2r = bass.AP(tensor=e2[:].tensor, offset=e2[:].offset,
                              ap=[list(e2[:].ap[0]), [0, 2], [1, SLABW]])
                nc.vector.tensor_mul(a01[:], e2r, wm_sb[:])
                A[s] = (a01, a01)
                A.pop(s - 3, None)

            def apply_pair(j):
                ap_ps = aps.tile([128, 130], F32)
                ops = [(d, m) for d in (0, 1, 2, -2, -1, 3, 4) for m in (0, 1)]
                for idx, (d, m) in enumerate(ops):
                    r = 4 * j + d
                    off = OFF_OF_D[d]
                    if r < 0 or r >= IMG:
                        src = wm_sb
                    else:
                        src = A[r // 4][m]
                    off += m * SLABW
                    rhs = _ap(v_sb, (r + 2) * VCH + m, [[2, 129]])
                    st = idx == 0
                    sp = idx == len(ops) - 1
                    if APPLY_COLSPLIT:
                        nc.tensor.matmul(ap_ps[0:64, 0:129],
                                         src[:, off:off + 64], rhs,
                                         start=st, stop=sp,
                                         skip_group_check=True)
                        nc.tensor.matmul(ap_ps[64:128, 0:129],
                                         src[:, off + 64:off + 128], rhs,
                                         start=st, stop=sp,
                                         skip_group_check=True,
                                         tile_position=(0, 64))
                    else:
                        nc.tensor.matmul(ap_ps[:, 0:129],
                                         src[:, off:off + 128], rhs,
                                         start=st, stop=sp,
                                         skip_group_check=True)
                den = dens.tile([128, 1], F32)
                nc.vector.tensor_add(den[:], ap_ps[:, 128:129], oob_sb[:])
                rec = dens.tile([128, 1], F32)
                nc.vector.reciprocal(rec[:], den[:])
                o_sb = outsb.tile([128, 128], F32)
                nc.scalar.activation(out=o_sb[:], in_=ap_ps[:, 0:128],
                                     func=COPY, scale=rec[:])
                nc.gpsimd.dma_start(out=out_d.ap()[j * 128:(j + 1) * 128, :],
                                    in_=o_sb[:])

            for i in range(4):
                make_y(i)
                if i < 3:
                    make_v_mms(i)
                    make_v_copy(i)
            for s in range(NPAIR):
                make_slab(s)
                if s + 3 < NPAIR:
                    make_v_mms(s + 3)
                if s >= 1:
                    apply_pair(s - 1)
                if s + 3 < NPAIR:
                    make_v_copy(s + 3)
            apply_pair(NPAIR - 1)

    nc.compile()
    return nc


_NC_CACHE = None


def kernel(x, w_q, w_k, w_v, row_emb, col_emb, mix_emb):
    global _NC_CACHE
    x = np.asarray(x, np.float32)
    w_q = np.asarray(w_q, np.float32)
    w_k = np.asarray(w_k, np.float32)
    w_v = np.asarray(w_v, np.float32)
    row_emb = np.asarray(row_emb, np.float32)
    col_emb = np.asarray(col_emb, np.float32)
    mix_emb = np.asarray(mix_emb, np.float32)

    G = (w_q.T @ w_k).astype(ml_dtypes.bfloat16)          # [64, 64]
    g2 = np.hstack([G, G])                                # [64, 128]
    wvt = np.vstack([w_v.T] * 2).astype(ml_dtypes.bfloat16)  # [128, 256]
    wpos = make_wpos(row_emb, col_emb, mix_emb)
    wmask = make_masks(wpos).reshape(128, 2 * SLABW).astype(ml_dtypes.bfloat16)
    oob = make_oob()

    xb = x.astype(ml_dtypes.bfloat16)                      # [B, 64, 128, 128]
    xe = np.ascontiguousarray(xb[:, :, 0::2, :])           # [B, 64, 64, 128]
    xo = np.ascontiguousarray(xb[:, :, 1::2, :])

    if _NC_CACHE is None:
        _NC_CACHE = build_nc()
    nc = _NC_CACHE

    in_maps = []
    for b in range(NCORES):
        in_maps.append({
            "xe": xe[b],
            "xo": xo[b],
            "g2": g2,
            "wvt": wvt,
            "wmask": wmask,
            "oob": oob,
        })
    res = run_bass_kernel_spmd(nc, in_maps, core_ids=list(range(NCORES)))
    out = np.stack([res.results[b]["out"].T.reshape(OC, HO, HO) for b in range(NCORES)])
    return out.astype(np.float32)
